# revision 1
# baseline (speedup 1.0000x reference)
"""Trainium2 Bass kernel: bipartite GNN message passing (BranchingGNN), 8-core SPMD.

Sharding: core k owns constraint rows [k*6250,(k+1)*6250) and variable rows
[k*12500,(k+1)*12500); each core processes all edges targeting its shard, so
messages need no cross-core reduction. Node tables live row-major in DRAM
(bf16 features in the first 128B of a 256B-strided row) and are re-broadcast
each phase by an AllGather of the updated shards.

Per phase (one message direction):
  - edges sorted by destination; each destination's run is split by source
    window (int16 gather reach) and padded to 4-edge slots; slots are packed
    into 128-edge tiles per (dst-block, window), streamed window-major.
  - dma_gather (custom emit: 128B rows at 256B stride) fetches source rows,
    128 edges per partition-tile, up to 7 tiles per call (SWDGE ring cap).
  - stage 1: fixed one-hot S_fix [128,32] reduces each tile to 32 slot sums;
    4 tiles packed into one PSUM [128,64] via PE tile_position.
  - stage 2: one-hot S2 [128,128] (DVE iota==pdst compare, -1 pads give zero
    rows) turns pack partials into the block's msgT [64,128] contribution,
    accumulated into an SBUF msg buffer.
  - update: relu(h_prevT + W.T @ msgT + b) in transposed layout (per-partition
    bias on ACT); PE transpose back to row-major, DMA to shard, AllGather.
"""
import sys

sys.path.insert(0, "/opt/trn_rl_repo")

import numpy as np
import ml_dtypes

import concourse.bass as bass
import concourse.bacc as bacc
import concourse.mybir as mybir
import concourse.tile as tile
from concourse.bass_utils import run_bass_kernel_spmd

# ---- problem constants
V, C, E = 100000, 50000, 1250000
VF, CF, H = 32, 32, 64
ROUNDS = 3
CORES = 8
P = 128
WSLOT = 4             # edges per slot
SLOTS = P // WSLOT    # 32 slot-sums per 128-edge tile
TPP = 4               # tiles per pack (128 partials)
TPC = 7               # tiles per gather call (SWDGE ring cap 64 descs)
ROWB = 128            # table row width in bf16 elems (64 data + 64 pad = 256B)

V_CORE, C_CORE = 12500, 6250          # real nodes per core
V_S, C_S = 12672, 6400                # shard rows (99 / 50 blocks)
NBU_V, NBU_C = 98, 49                 # updated blocks (last block stays zero)
RV, RC = CORES * V_S, CORES * C_S     # 101376 / 51200 table rows
VWIN, CWIN = 4, 2                     # source windows (2 shards / 4 shards)
VWROWS, CWROWS = 2 * V_S, 4 * C_S     # 25344 / 25600 rows per window
VDUMMY, CDUMMY = 12544, 6272          # window-local zero row

BF16 = mybir.dt.bfloat16
F32 = mybir.dt.float32
I16 = mybir.dt.int16
BF = ml_dtypes.bfloat16


def _win_local(src, n_core, shard, per_win):
    """global node id -> (window, window-local table row)."""
    w = src // (per_win * n_core)
    local = (src % (per_win * n_core)) // n_core * shard + src % n_core
    return w, local


def _prep_direction(dst, src, n_dst_core, nblk_upd, nwin, src_core, src_shard,
                    dummy_row):
    """Metadata for one direction. Returns (idx16 [CORES,128,8*Ttot],
    pdst [CORES,128,Ptot], Tbw [nwin, nblk_upd])."""
    dst = np.asarray(dst, np.int64)
    src = np.asarray(src, np.int64)
    E_ = dst.size
    per_win_ids = src_core * (CORES // nwin)      # real ids per window

    core_of = dst // n_dst_core
    d_loc = dst % n_dst_core
    b_of = d_loc // P
    w_of = src // per_win_ids
    widx = (src % per_win_ids) // src_core * src_shard + src % src_core

    # sort edges by (window, core, block, dst)
    key = ((w_of * CORES + core_of) * nblk_upd + b_of) * P + d_loc % P
    order = np.argsort(key, kind="stable")
    ks = key[order]
    widx_s = widx[order]

    # run ranks within each (w, core, b, dst)
    counts = np.bincount(ks, minlength=nwin * CORES * nblk_upd * P)
    run_start = np.zeros(counts.size + 1, np.int64)
    run_start[1:] = np.cumsum(counts)
    rank = np.arange(E_, dtype=np.int64) - run_start[ks]

    slot_cnt = -(-counts // WSLOT)                                  # per key
    sc4 = slot_cnt.reshape(nwin, CORES, nblk_upd, P)
    blk_slots = sc4.sum(-1)                                         # [w,core,b]
    Tbw = np.maximum((-(-blk_slots // SLOTS)).max(1), 1)            # [w, b]

    # slot offset of each key within its (w, core, b) group
    sc_cum = np.cumsum(sc4, -1) - sc4                                # excl
    # tile base of (w, b): window-major, blocks in order (same every core)
    tiles_w = Tbw.sum(1)                                             # [w]
    win_base = np.zeros(nwin + 1, np.int64)
    win_base[1:] = np.cumsum(tiles_w)
    blk_base = np.cumsum(Tbw, 1) - Tbw                               # [w, b]
    Ttot = int(tiles_w.sum())

    w_s = ks // (CORES * nblk_upd * P)
    rem = ks % (CORES * nblk_upd * P)
    c_s = rem // (nblk_upd * P)
    b_s = rem % (nblk_upd * P) // P

    slot_pos = (win_base[w_s] + blk_base[w_s, b_s]) * SLOTS \
        + sc_cum.reshape(-1)[ks] + rank // WSLOT
    epos = slot_pos * WSLOT + rank % WSLOT

    idx16 = np.full((CORES, Ttot * P), dummy_row, np.int16)
    idx16[c_s, epos] = widx_s.astype(np.int16)

    # packs: per (w, b): ceil(Tbw/4); pdst flat slot -> pack/partial
    Pbw = -(-Tbw // TPP)                                             # [w, b]
    packs_w = Pbw.sum(1)
    pwin_base = np.zeros(nwin + 1, np.int64)
    pwin_base[1:] = np.cumsum(packs_w)
    pblk_base = np.cumsum(Pbw, 1) - Pbw
    Ptot = int(packs_w.sum())

    # slot position within its (w,b) group:
    s_in_blk = slot_pos - (win_base[w_s] + blk_base[w_s, b_s]) * SLOTS
    pack_of = pwin_base[w_s] + pblk_base[w_s, b_s] + s_in_blk // P
    part_of = s_in_blk % P
    pdst = np.full((CORES, Ptot, P), -1.0, np.float32)
    pdst[c_s, pack_of, part_of] = (ks % P).astype(np.float32)
    pdst = pdst.transpose(0, 2, 1).copy()                            # [CORES,128,Ptot]

    # idx16 -> dma_gather wrap layout [CORES, 128, 8*Ttot]
    packed = np.zeros((CORES, P, Ttot * 8), np.int16)
    for k in range(CORES):
        a = idx16[k].reshape(-1, 16).T                               # [16, Ttot*8]
        packed[k] = np.tile(a, (8, 1))
    return packed, pdst, Tbw.astype(int)



def _dma_gather_raw(gp, out_ap, in_ap, idxs_ap, num_idxs, elem_size, elem_step,
                    queue_num=0):
    """dma_gather (non-transpose, HBM source) allowing 128B rows at 256B stride."""
    from concourse import ap_utils
    gp._assert_queue_num(queue_num)
    assert idxs_ap.dtype == mybir.dt.int16
    assert in_ap.dtype == out_ap.dtype
    assert ap_utils.ap_is_contiguous(in_ap.ap[1:])
    assert ap_utils.ap_is_contiguous(out_ap.ap[1:])
    assert ap_utils.ap_is_contiguous(idxs_ap.ap[1:])
    assert in_ap.ap[-1][1] == out_ap.ap[-1][1] == elem_size
    assert out_ap.ap[0][1] * out_ap.ap[1][1] == num_idxs and num_idxs % 128 == 0
    assert in_ap.ap[0][0] == elem_step
    stride_bytes = elem_step * mybir.dt.size(in_ap.dtype)
    stride_bytes_256 = stride_bytes // 256
    assert stride_bytes_256 * 256 == stride_bytes and stride_bytes_256 < 256
    _in_ap = gp.lower_ap_dma(in_ap, for_custom_bir_dma=True)
    _idxs_ap = gp.lower_ap(idxs_ap)
    _out_ap = gp.lower_ap(out_ap)
    return gp.add_instruction(
        mybir.InstDMAGatherAnt(
            name=gp.bass.get_next_instruction_name(),
            ins=[*_in_ap, _idxs_ap, gp.lower_val_access(gp.to_reg(num_idxs))],
            outs=[_out_ap],
            transpose=False, num_idxs=num_idxs, elem_size=elem_size,
            stride_bytes_256=stride_bytes_256, gen_mode=0, single_packet=True,
            queue_num=queue_num, sbuf_tokens_per_rank=0,
            sbuf_free_dim_per_rank=0, sbuf_free_dim_pad_per_rank=0,
            sbuf_byte_offset=0))

def _build(Tbw_c, Tbw_v, b_score_val):
    """Build the shared SPMD program."""

    Tt_c, Tt_v = int(Tbw_c.sum()), int(Tbw_v.sum())
    Pk_c = int((-(-Tbw_c // TPP)).sum())
    Pk_v = int((-(-Tbw_v // TPP)).sum())

    nc = bacc.Bacc("TRN2", target_bir_lowering=False, num_devices=CORES,
                   num_swdge_queues=4)
    AluOp = mybir.AluOpType
    Act = mybir.ActivationFunctionType

    def ein(name, shape, dtype):
        return nc.dram_tensor(name, shape, dtype, kind="ExternalInput")

    vfT = ein("vfT", [VF, V_S], F32)
    cfT = ein("cfT", [CF, C_S], F32)
    wvar = ein("wvar", [VF, H], F32)
    wcon = ein("wcon", [CF, H], F32)
    wv2c = ein("wv2c", [H, H], F32)
    wc2v = ein("wc2v", [H, H], F32)
    wsco = ein("wsco", [H, 1], BF16)
    bvar = ein("bvar", [H, 1], F32)
    bcon = ein("bcon", [H, 1], F32)
    bv2c = ein("bv2c", [H, 1], F32)
    bc2v = ein("bc2v", [H, 1], F32)
    idx_v2c_d = ein("idx_v2c", [P, Tt_c * 8], I16)
    idx_c2v_d = ein("idx_c2v", [P, Tt_v * 8], I16)
    pdst_v2c_d = ein("pdst_v2c", [P, Pk_c], F32)
    pdst_c2v_d = ein("pdst_c2v", [P, Pk_v], F32)
    sfix_d = ein("sfix", [P, SLOTS], BF16)
    iota_d = ein("iota", [P, P], F32)
    ident_d = ein("ident", [H, H], BF16)
    scores_out = nc.dram_tensor("scores", [V_S], F32, kind="ExternalOutput")

    with tile.TileContext(nc) as tc:
        with (
            tc.tile_pool(name="const", bufs=1) as cpool,
            tc.tile_pool(name="state", bufs=1) as spool,
            tc.tile_pool(name="dram", bufs=1, space="DRAM") as dpool,
            tc.tile_pool(name="gpool", bufs=14) as gpool,
            tc.tile_pool(name="parts", bufs=3) as parts_pool,
            tc.tile_pool(name="s2p", bufs=3) as s2_pool,
            tc.tile_pool(name="rowp", bufs=2) as row_pool,
            tc.tile_pool(name="ps_pack", bufs=3, space="PSUM") as ps_pack,
            tc.tile_pool(name="ps_s2o", bufs=3, space="PSUM") as ps_s2o,
            tc.tile_pool(name="ps_misc", bufs=2, space="PSUM") as ps_misc,
        ):
            def load_const(name, dram, shape, dtype):
                t = cpool.tile(shape, dtype, name=name)
                nc.sync.dma_start(out=t[:], in_=dram[:])
                return t

            sfix_sb = load_const("sfix_sb", sfix_d, [P, SLOTS], BF16)
            iota_sb = load_const("iota_sb", iota_d, [P, P], F32)
            ident_sb = load_const("ident_sb", ident_d, [H, H], BF16)
            wvar_sb = load_const("wvar_sb", wvar, [VF, H], F32)
            wcon_sb = load_const("wcon_sb", wcon, [CF, H], F32)
            wv2c_sb = load_const("wv2c_sb", wv2c, [H, H], F32)
            wc2v_sb = load_const("wc2v_sb", wc2v, [H, H], F32)
            wsco_sb = load_const("wsco_sb", wsco, [H, 1], BF16)
            bvar_sb = load_const("bvar_sb", bvar, [H, 1], F32)
            bcon_sb = load_const("bcon_sb", bcon, [H, 1], F32)
            bv2c_sb = load_const("bv2c_sb", bv2c, [H, 1], F32)
            bc2v_sb = load_const("bc2v_sb", bc2v, [H, 1], F32)
            idx_sh = cpool.tile([P, max(Tt_c, Tt_v) * 8], I16, name="idx_sh")
            pdst_v2c_sb = load_const("pdst_v2c_sb", pdst_v2c_d, [P, Pk_c], F32)
            pdst_c2v_sb = load_const("pdst_c2v_sb", pdst_c2v_d, [P, Pk_v], F32)

            szero_sb = cpool.tile([P, SLOTS], BF16, name="szero_sb")
            nc.vector.memset(szero_sb[:], 0.0)
            gdummy_sb = cpool.tile([P, H], BF16, name="gdummy_sb")
            nc.vector.memset(gdummy_sb[:], 0.0)
            zrow_sb = cpool.tile([P, ROWB], BF16, name="zrow_sb")
            nc.vector.memset(zrow_sb[:], 0.0)

            hvT = spool.tile([H, V_S], BF16, name="hvT")
            hcT = spool.tile([H, C_S], BF16, name="hcT")
            macc_sh = spool.tile([H, NBU_V * P], F32, name="macc_sh")
            tabs_v = [dpool.tile([RV, ROWB], BF16, name=f"tab_v{i}",
                                 addr_space="Shared", tag=f"tab_v{i}")
                      for i in range(ROUNDS)]
            tabs_c = [dpool.tile([RC, ROWB], BF16, name=f"tab_c{i}",
                                 addr_space="Shared", tag=f"tab_c{i}")
                      for i in range(ROUNDS)]
            agin_v = dpool.tile([V_S, ROWB], BF16, name="agin_v")
            agin_c = dpool.tile([C_S, ROWB], BF16, name="agin_c")

            # zero the shard tail (dummy rows shipped by every AllGather)
            nc.sync.dma_start(out=agin_v[NBU_V * P:V_S, :], in_=zrow_sb[:])
            nc.sync.dma_start(out=agin_c[NBU_C * P:C_S, :], in_=zrow_sb[:])

            # ---- initial embeddings hT = relu(W.T @ featT + b)
            def emit_init(featT_dram, fdim, n_s, w_sb, b_sb, hT):
                with tc.tile_pool(name="initp", bufs=2) as ipool:
                    c0 = 0
                    while c0 < n_s:
                        w = min(512, n_s - c0)
                        fch = ipool.tile([fdim, 512], F32, name="fch", tag="fch")
                        nc.sync.dma_start(out=fch[:, :w],
                                          in_=featT_dram[:, c0:c0 + w])
                        psi = ps_misc.tile([H, 512], F32, name="psi", tag="misc")
                        nc.tensor.matmul(out=psi[:, :w], lhsT=w_sb[:],
                                         rhs=fch[:, :w], start=True, stop=True)
                        nc.scalar.activation(out=hT[:, c0:c0 + w], in_=psi[:, :w],
                                             func=Act.Relu, bias=b_sb[:])
                        c0 += w

            emit_init(vfT, VF, V_S, wvar_sb, bvar_sb, hvT)
            emit_init(cfT, CF, C_S, wcon_sb, bcon_sb, hcT)

            def emit_writeback(hT, nblk, agin, tab, nrows):
                rstage = row_pool.tile([P, NBU_V * H], BF16, name="rstage",
                                       tag="rstage")
                for b in range(nblk):
                    psr = ps_misc.tile([P, H], BF16, name="psr", tag="misc")
                    nc.tensor.transpose(out=psr[:], in_=hT[:, b * P:(b + 1) * P],
                                        identity=ident_sb[:])
                    nc.vector.tensor_copy(out=rstage[:, b * H:(b + 1) * H],
                                          in_=psr[:])
                nc.sync.dma_start(
                    out=agin[0:nblk * P, 0:H].rearrange("(b p) f -> p b f", p=P),
                    in_=rstage[:, :nblk * H].rearrange("p (b f) -> p b f", f=H))
                nc.gpsimd.collective_compute(
                    "AllGather", mybir.AluOpType.bypass,
                    replica_groups=[list(range(CORES))],
                    ins=[agin[:]], outs=[tab[:]])

            emit_writeback(hvT, NBU_V, agin_v, tabs_v[0], RV)

            # ---- one message-passing phase
            def emit_phase(tab_src, wrows, idx_dram, ncols, pdst_sb, Tbw, nblk,
                           macc, hT, W_sb, b_sb, writeback):
                nwin = Tbw.shape[0]
                idx_sb = idx_sh
                nc.sync.dma_start(out=idx_sb[:, :ncols], in_=idx_dram[:])
                nc.vector.memset(macc[:, :nblk * P], 0.0)
                g_tiles = {}

                def g_ap(w, base_w, tw):
                    cidx = tw // TPC
                    if (w, cidx) not in g_tiles:
                        ntile = min(TPC, int(Tbw[w].sum()) - cidx * TPC)
                        g = gpool.tile([P, TPC, H], BF16, name="g", tag="g")
                        _dma_gather_raw(
                            nc.gpsimd, g[:, :ntile, :],
                            tab_src[w * wrows:(w + 1) * wrows, 0:H],
                            idx_sb[:, (base_w + cidx * TPC) * 8:
                                   (base_w + cidx * TPC + ntile) * 8],
                            num_idxs=ntile * P, elem_size=H, elem_step=ROWB,
                            queue_num=cidx % 4)
                        g_tiles[(w, cidx)] = g
                    return g_tiles[(w, cidx)][:, tw % TPC, :]

                win_base = np.concatenate([[0], np.cumsum(Tbw.sum(1))])
                pk = 0
                for w in range(nwin):
                    tw = 0
                    for b in range(nblk):
                        npk = -(-int(Tbw[w, b]) // TPP)
                        for p_ in range(npk):
                            psp = ps_pack.tile([P, H], F32, name="psp", tag="psp")
                            for j in range(TPP):
                                t = TPP * p_ + j
                                if t < Tbw[w, b]:
                                    lhs = sfix_sb[:]
                                    rhs = g_ap(w, int(win_base[w]), tw + t)
                                else:
                                    lhs, rhs = szero_sb[:], gdummy_sb[:]
                                nc.tensor.matmul(
                                    out=psp[j * SLOTS:(j + 1) * SLOTS, :],
                                    lhsT=lhs, rhs=rhs, start=True, stop=True,
                                    tile_position=(0, j * SLOTS))
                            parts = parts_pool.tile([P, H], F32, name="parts",
                                                    tag="parts")
                            nc.vector.tensor_copy(out=parts[:], in_=psp[:])
                            s2 = s2_pool.tile([P, P], F32, name="s2", tag="s2")
                            nc.vector.tensor_tensor(
                                out=s2[:], in0=iota_sb[:],
                                in1=pdst_sb[:, pk:pk + 1].to_broadcast([P, P]),
                                op=AluOp.is_equal)
                            pso = ps_s2o.tile([H, P], F32, name="pso", tag="pso")
                            nc.tensor.matmul(out=pso[:], lhsT=parts[:], rhs=s2[:],
                                             start=True, stop=True)
                            nc.vector.tensor_tensor(
                                out=macc[:, b * P:(b + 1) * P],
                                in0=macc[:, b * P:(b + 1) * P], in1=pso[:],
                                op=AluOp.add)
                            pk += 1
                        tw += int(Tbw[w, b])

                for b in range(nblk):
                    psu = ps_misc.tile([H, P], F32, name="psu", tag="misc")
                    nc.tensor.matmul(out=psu[:], lhsT=W_sb[:],
                                     rhs=macc[:, b * P:(b + 1) * P],
                                     start=True, stop=True)
                    tmp = s2_pool.tile([H, P], F32, name="tmp", tag="tmp")
                    nc.vector.tensor_tensor(out=tmp[:], in0=psu[:],
                                            in1=hT[:, b * P:(b + 1) * P],
                                            op=AluOp.add)
                    nc.scalar.activation(out=hT[:, b * P:(b + 1) * P], in_=tmp[:],
                                         func=Act.Relu, bias=b_sb[:])
                if writeback is not None:
                    writeback()

            for r in range(ROUNDS):
                emit_phase(tabs_v[r], VWROWS, idx_v2c_d, Tt_c * 8, pdst_v2c_sb,
                           Tbw_c, NBU_C, macc_sh, hcT, wv2c_sb, bv2c_sb,
                           lambda r=r: emit_writeback(hcT, NBU_C, agin_c,
                                                      tabs_c[r], RC))
                last = r == ROUNDS - 1
                emit_phase(tabs_c[r], CWROWS, idx_c2v_d, Tt_v * 8, pdst_c2v_sb,
                           Tbw_v, NBU_V, macc_sh, hvT, wc2v_sb, bc2v_sb,
                           None if last else
                           (lambda r=r: emit_writeback(hvT, NBU_V, agin_v,
                                                       tabs_v[r + 1], RV)))

            # ---- scores = h_var @ w_score + b_score (shard)
            c0 = 0
            while c0 < V_S:
                w = min(512, V_S - c0)
                pss = ps_misc.tile([1, 512], F32, name="pss", tag="misc")
                nc.tensor.matmul(out=pss[:, :w], lhsT=wsco_sb[:],
                                 rhs=hvT[:, c0:c0 + w], start=True, stop=True)
                sch = s2_pool.tile([1, 512], F32, name="sch", tag="sch")
                nc.vector.tensor_scalar(
                    out=sch[:, :w], in0=pss[:, :w],
                    scalar1=float(b_score_val), scalar2=None, op0=AluOp.add)
                nc.sync.dma_start(out=scores_out[None, c0:c0 + w],
                                  in_=sch[0:1, :w])
                c0 += w

    nc.compile()
    return nc


_CACHE = {}


def kernel(**inputs):
    var_feat = np.asarray(inputs["var_feat"], np.float32)
    constr_feat = np.asarray(inputs["constr_feat"], np.float32)
    var_idx = np.asarray(inputs["var_idx"]).astype(np.int64)
    constr_idx = np.asarray(inputs["constr_idx"]).astype(np.int64)
    b_score_val = float(np.asarray(inputs["b_score"]).reshape(-1)[0])

    key = (var_idx.tobytes(), constr_idx.tobytes())
    if key in _CACHE:
        nc, idx_v, pdst_v, idx_c, pdst_c = _CACHE[key]
    else:
        # v2c: dst=constr, src=var
        idx_v, pdst_v, Tbw_c = _prep_direction(
            constr_idx, var_idx, C_CORE, NBU_C, VWIN, V_CORE, V_S, VDUMMY)
        # c2v: dst=var, src=constr
        idx_c, pdst_c, Tbw_v = _prep_direction(
            var_idx, constr_idx, V_CORE, NBU_V, CWIN, C_CORE, C_S, CDUMMY)
        nc = _build(Tbw_c, Tbw_v, b_score_val)
        _CACHE[key] = (nc, idx_v, pdst_v, idx_c, pdst_c)

    iota = np.broadcast_to(np.arange(P, dtype=np.float32), (P, P)).copy()
    sfix = (np.arange(P)[:, None] // WSLOT == np.arange(SLOTS)[None, :]).astype(BF)
    ident = np.eye(H, dtype=np.float32).astype(BF)

    vf_pad = np.zeros((CORES, V_S, VF), np.float32)
    vf_pad[:, :V_CORE] = var_feat.reshape(CORES, V_CORE, VF)
    cf_pad = np.zeros((CORES, C_S, CF), np.float32)
    cf_pad[:, :C_CORE] = constr_feat.reshape(CORES, C_CORE, CF)

    common = dict(
        wvar=np.ascontiguousarray(inputs["W_var"], dtype=np.float32),
        wcon=np.ascontiguousarray(inputs["W_con"], dtype=np.float32),
        wv2c=np.ascontiguousarray(inputs["W_v2c"], dtype=np.float32),
        wc2v=np.ascontiguousarray(inputs["W_c2v"], dtype=np.float32),
        wsco=np.ascontiguousarray(inputs["W_score"], dtype=np.float32).astype(BF),
        bvar=np.ascontiguousarray(inputs["b_var"], dtype=np.float32).reshape(H, 1),
        bcon=np.ascontiguousarray(inputs["b_con"], dtype=np.float32).reshape(H, 1),
        bv2c=np.ascontiguousarray(inputs["b_v2c"], dtype=np.float32).reshape(H, 1),
        bc2v=np.ascontiguousarray(inputs["b_c2v"], dtype=np.float32).reshape(H, 1),
        sfix=sfix, iota=iota, ident=ident,
    )
    in_maps = []
    for k in range(CORES):
        m = dict(common)
        m["vfT"] = np.ascontiguousarray(vf_pad[k].T)
        m["cfT"] = np.ascontiguousarray(cf_pad[k].T)
        m["idx_v2c"] = idx_v[k]
        m["pdst_v2c"] = pdst_v[k]
        m["idx_c2v"] = idx_c[k]
        m["pdst_c2v"] = pdst_c[k]
        in_maps.append(m)

    res = run_bass_kernel_spmd(nc, in_maps, list(range(CORES)))
    scores = np.concatenate([res.results[k]["scores"].reshape(-1)[:V_CORE]
                             for k in range(CORES)])
    return scores.astype(np.float32)



# revision 5
# speedup vs baseline: 3.2110x; 3.2110x over previous
"""Trainium2 Bass kernel: bipartite GNN message passing (BranchingGNN), 8-core SPMD.

Sharding: core k owns constraint rows [k*6250,(k+1)*6250) and variable rows
[k*12500,(k+1)*12500); each core processes all edges targeting its shard, so
messages need no cross-core reduction. Node tables live row-major in DRAM
(bf16 features in the first 128B of a 256B-strided row) and are re-broadcast
each phase by an AllGather of the updated shards.

Per phase (one message direction):
  - edges sorted by (dst-block, src-window, dst); per (block, window) group the
    raw edge list is cut into 128-edge tiles (up to 7 tiles per gather call).
    No slot padding and no dummy-row fetches: per-core shortfalls are trailing
    -1 indices, which the SWDGE ucode trims before descriptor generation.
  - dma_gather (custom emit: 128B rows at 256B stride) fetches source rows
    row-major [128 edges, 64] bf16.
  - per tile: DVE is_equal(iota, pdst) builds a one-hot S [128,128] bf16
    (pdst = within-block dst of each edge, -1 for pads -> zero row); one PE
    matmul (lhsT=g, rhs=S) accumulates the tile's segment-sum contribution
    into the block's PSUM [64,128] msgT.
  - per block: relu(h_prevT + W.T @ msgT + b) in transposed layout, PE
    transpose back to row-major into the writeback stage; after all blocks one
    DMA + AllGather republishes the updated shard.
"""
import sys

sys.path.insert(0, "/opt/trn_rl_repo")

import numpy as np
import ml_dtypes

import concourse.bass as bass
import concourse.bacc as bacc
import concourse.mybir as mybir
import concourse.tile as tile
from concourse.bass_utils import run_bass_kernel_spmd

# ---- problem constants
V, C, E = 100000, 50000, 1250000
VF, CF, H = 32, 32, 64
ROUNDS = 3
CORES = 8
P = 128
TPC = 7               # tiles per gather call (SWDGE ring cap)
ROWB = 128            # table row width in bf16 elems (64 data + 64 pad = 256B)

V_CORE, C_CORE = 12500, 6250          # real nodes per core
V_S, C_S = 12672, 6400                # shard rows (99 / 50 blocks)
NBU_V, NBU_C = 98, 49                 # updated blocks (last block stays zero)
RV, RC = CORES * V_S, CORES * C_S     # 101376 / 51200 table rows
VWIN, CWIN = 4, 2                     # source windows (int16 gather reach)
VWROWS, CWROWS = 2 * V_S, 4 * C_S     # 25344 / 25600 rows per window

BF16 = mybir.dt.bfloat16
F32 = mybir.dt.float32
I16 = mybir.dt.int16
BF = ml_dtypes.bfloat16


def _prep_direction(dst, src, n_dst_core, nblk, nwin, src_core, src_shard):
    """Per-direction metadata. Edges sorted by (core, block, window, dst);
    per (block, window) group cut into 128-edge tiles, per-core shortfall
    filled with trailing -1.

    Returns (idx_wrapped [CORES,128,T*8] int16, pdst [CORES,128,T] bf16,
    Tbw [nblk, nwin] int tile counts)."""
    dst = np.asarray(dst, np.int64)
    src = np.asarray(src, np.int64)
    per_win_ids = src_core * (CORES // nwin)

    core_of = dst // n_dst_core
    d_loc = dst % n_dst_core
    b_of = d_loc // P
    w_of = src // per_win_ids
    widx = (src % per_win_ids) // src_core * src_shard + src % src_core

    # sort by (core, block, window, dst)
    key = ((core_of * nblk + b_of) * nwin + w_of) * P + d_loc % P
    order = np.argsort(key, kind="stable")
    ks = key[order]
    widx_s = widx[order]
    pd_s = (d_loc % P)[order]

    counts = np.bincount(ks // P, minlength=CORES * nblk * nwin) \
        .reshape(CORES, nblk, nwin)
    Tbw = -(-counts.max(0) // P)                     # [nblk, nwin] max tiles
    Tbw = np.maximum(Tbw, (counts.max(0) > 0))       # 0 only if empty all cores

    grp_base = np.zeros((nblk, nwin), np.int64)      # tile base of group
    flat = Tbw.reshape(-1)
    grp_base.reshape(-1)[1:] = np.cumsum(flat)[:-1]
    Ttot = int(flat.sum())

    # position of each edge within its (core, b, w) group
    gk = ks // P                                     # core,b,w group id
    gcounts = np.bincount(gk, minlength=CORES * nblk * nwin)
    gstart = np.zeros(gcounts.size + 1, np.int64)
    gstart[1:] = np.cumsum(gcounts)
    rank = np.arange(dst.size, dtype=np.int64) - gstart[gk]

    c_s = gk // (nblk * nwin)
    bw = gk % (nblk * nwin)
    epos = grp_base.reshape(-1)[bw] * P + rank

    idx16 = np.full((CORES, Ttot * P), -1, np.int16)
    idx16[c_s, epos] = widx_s.astype(np.int16)

    # negative idxs crash the gather ucode, so every pad slot must carry a
    # valid row. Cycle each call's pads over that call's own valid idxs
    # (rows already being fetched, spread — no hot row); pdst stays -1 so
    # pads contribute nothing to the segment sum.
    flatT = Tbw.reshape(-1)
    flat_base = grp_base.reshape(-1)
    gcnt = counts  # [CORES, nblk, nwin]
    for bwi in np.nonzero(flatT > 0)[0]:
        base, T = int(flat_base[bwi]), int(flatT[bwi])
        b_, w_ = bwi // nwin, bwi % nwin
        for j0 in range(0, T, TPC):
            clen = min(TPC, T - j0) * P
            cpos = (base + j0) * P
            for k in range(CORES):
                valid = min(max(int(gcnt[k, b_, w_]) - j0 * P, 0), clen)
                if valid == clen:
                    continue
                if valid > 0:
                    src_slice = idx16[k, cpos:cpos + valid]
                    npad = clen - valid
                    reps = -(-npad // valid)
                    idx16[k, cpos + valid:cpos + clen] = \
                        np.tile(src_slice, reps)[:npad]
                else:
                    gbase = int(flat_base[bwi]) * P
                    gval = min(int(gcnt[k, b_, w_]), clen)
                    if gval == 0:
                        idx16[k, cpos:cpos + clen] = 0
                    else:
                        reps = -(-clen // gval)
                        idx16[k, cpos:cpos + clen] = \
                            np.tile(idx16[k, gbase:gbase + gval], reps)[:clen]
    pdst = np.full((CORES, Ttot * P), -1.0, np.float32)
    pdst[c_s, epos] = pd_s.astype(np.float32)
    pdst = pdst.reshape(CORES, Ttot, P).transpose(0, 2, 1)  # [CORES,128,T]

    packed = np.zeros((CORES, P, Ttot * 8), np.int16)
    for k in range(CORES):
        a = idx16[k].reshape(-1, 16).T               # [16, Ttot*8]
        packed[k] = np.tile(a, (8, 1))
    return packed, pdst.astype(BF), Tbw.astype(int)


def _dma_gather_raw(gp, out_ap, in_ap, idxs_ap, num_idxs, elem_size, elem_step,
                    queue_num=0):
    """dma_gather (non-transpose, HBM source) allowing 128B rows at 256B stride."""
    from concourse import ap_utils
    gp._assert_queue_num(queue_num)
    assert idxs_ap.dtype == mybir.dt.int16
    assert in_ap.dtype == out_ap.dtype
    assert ap_utils.ap_is_contiguous(in_ap.ap[1:])
    assert ap_utils.ap_is_contiguous(out_ap.ap[1:])
    assert ap_utils.ap_is_contiguous(idxs_ap.ap[1:])
    assert in_ap.ap[-1][1] == out_ap.ap[-1][1] == elem_size
    assert out_ap.ap[0][1] * out_ap.ap[1][1] == num_idxs and num_idxs % 128 == 0
    assert in_ap.ap[0][0] == elem_step
    stride_bytes = elem_step * mybir.dt.size(in_ap.dtype)
    stride_bytes_256 = stride_bytes // 256
    assert stride_bytes_256 * 256 == stride_bytes and stride_bytes_256 < 256
    _in_ap = gp.lower_ap_dma(in_ap, for_custom_bir_dma=True)
    _idxs_ap = gp.lower_ap(idxs_ap)
    _out_ap = gp.lower_ap(out_ap)
    return gp.add_instruction(
        mybir.InstDMAGatherAnt(
            name=gp.bass.get_next_instruction_name(),
            ins=[*_in_ap, _idxs_ap, gp.lower_val_access(gp.to_reg(num_idxs))],
            outs=[_out_ap],
            transpose=False, num_idxs=num_idxs, elem_size=elem_size,
            stride_bytes_256=stride_bytes_256, gen_mode=0, single_packet=True,
            queue_num=queue_num, sbuf_tokens_per_rank=0,
            sbuf_free_dim_per_rank=0, sbuf_free_dim_pad_per_rank=0,
            sbuf_byte_offset=0))


def _build(Tbw_c, Tbw_v, b_score_val):
    """Build the shared SPMD program."""

    Tt_c, Tt_v = int(Tbw_c.sum()), int(Tbw_v.sum())

    nc = bacc.Bacc("TRN2", target_bir_lowering=False, num_devices=CORES,
                   num_swdge_queues=4)
    AluOp = mybir.AluOpType
    Act = mybir.ActivationFunctionType

    def ein(name, shape, dtype):
        return nc.dram_tensor(name, shape, dtype, kind="ExternalInput")

    vfT = ein("vfT", [VF, V_S], F32)
    cfT = ein("cfT", [CF, C_S], F32)
    wvar = ein("wvar", [VF, H], F32)
    wcon = ein("wcon", [CF, H], F32)
    wv2c = ein("wv2c", [H, H], F32)
    wc2v = ein("wc2v", [H, H], F32)
    wsco = ein("wsco", [H, 1], BF16)
    bvar = ein("bvar", [H, 1], F32)
    bcon = ein("bcon", [H, 1], F32)
    bv2c = ein("bv2c", [H, 1], F32)
    bc2v = ein("bc2v", [H, 1], F32)
    idx_v2c_d = ein("idx_v2c", [P, Tt_c * 8], I16)
    idx_c2v_d = ein("idx_c2v", [P, Tt_v * 8], I16)
    pdst_v2c_d = ein("pdst_v2c", [P, Tt_c], BF16)
    pdst_c2v_d = ein("pdst_c2v", [P, Tt_v], BF16)
    iota_d = ein("iota", [P, P], BF16)
    ident_d = ein("ident", [H, H], BF16)
    scores_out = nc.dram_tensor("scores", [V_S], F32, kind="ExternalOutput")

    with tile.TileContext(nc) as tc:
        with (
            tc.tile_pool(name="const", bufs=1) as cpool,
            tc.tile_pool(name="state", bufs=1) as spool,
            tc.tile_pool(name="dram", bufs=1, space="DRAM") as dpool,
            tc.tile_pool(name="gpool", bufs=24) as gpool,
            tc.tile_pool(name="s2p", bufs=6) as s2_pool,
            tc.tile_pool(name="msgp", bufs=3) as msg_pool,
            tc.tile_pool(name="rowp", bufs=2) as row_pool,
            tc.tile_pool(name="ps_acc", bufs=4, space="PSUM") as ps_acc,
            tc.tile_pool(name="ps_upd", bufs=2, space="PSUM") as ps_upd,
            tc.tile_pool(name="ps_misc", bufs=2, space="PSUM") as ps_misc,
        ):
            def load_const(name, dram, shape, dtype):
                t = cpool.tile(shape, dtype, name=name)
                nc.sync.dma_start(out=t[:], in_=dram[:])
                return t

            iota_sb = load_const("iota_sb", iota_d, [P, P], BF16)
            ident_sb = load_const("ident_sb", ident_d, [H, H], BF16)
            wvar_sb = load_const("wvar_sb", wvar, [VF, H], F32)
            wcon_sb = load_const("wcon_sb", wcon, [CF, H], F32)
            wv2c_sb = load_const("wv2c_sb", wv2c, [H, H], F32)
            wc2v_sb = load_const("wc2v_sb", wc2v, [H, H], F32)
            wsco_sb = load_const("wsco_sb", wsco, [H, 1], BF16)
            bvar_sb = load_const("bvar_sb", bvar, [H, 1], F32)
            bcon_sb = load_const("bcon_sb", bcon, [H, 1], F32)
            bv2c_sb = load_const("bv2c_sb", bv2c, [H, 1], F32)
            bc2v_sb = load_const("bc2v_sb", bc2v, [H, 1], F32)
            idx_v2c_sb = load_const("idx_v2c_sb", idx_v2c_d, [P, Tt_c * 8], I16)
            idx_c2v_sb = load_const("idx_c2v_sb", idx_c2v_d, [P, Tt_v * 8], I16)
            pdst_v2c_sb = load_const("pdst_v2c_sb", pdst_v2c_d, [P, Tt_c], BF16)
            pdst_c2v_sb = load_const("pdst_c2v_sb", pdst_c2v_d, [P, Tt_v], BF16)

            zrow_sb = cpool.tile([P, ROWB], BF16, name="zrow_sb")
            nc.vector.memset(zrow_sb[:], 0.0)

            # pre-zero gather buffers once: stale contents stay finite, so
            # S's zero rows always multiply finite values (no NaN*0).
            for _ in range(24):
                gz = gpool.tile([P, TPC, H], BF16, name="g", tag="g")
                nc.vector.memset(gz[:], 0.0)

            hvT = spool.tile([H, V_S], BF16, name="hvT")
            hcT = spool.tile([H, C_S], BF16, name="hcT")
            tabs_v = [dpool.tile([RV, ROWB], BF16, name=f"tab_v{i}",
                                 addr_space="Shared", tag=f"tab_v{i}")
                      for i in range(ROUNDS)]
            tabs_c = [dpool.tile([RC, ROWB], BF16, name=f"tab_c{i}",
                                 addr_space="Shared", tag=f"tab_c{i}")
                      for i in range(ROUNDS)]
            agin_v = dpool.tile([V_S, ROWB], BF16, name="agin_v")
            agin_c = dpool.tile([C_S, ROWB], BF16, name="agin_c")

            # zero the shard tail (pad rows shipped by every AllGather)
            nc.sync.dma_start(out=agin_v[NBU_V * P:V_S, :], in_=zrow_sb[:])
            nc.sync.dma_start(out=agin_c[NBU_C * P:C_S, :], in_=zrow_sb[:])

            # ---- initial embeddings hT = relu(W.T @ featT + b)
            def emit_init(featT_dram, fdim, n_s, w_sb, b_sb, hT):
                with tc.tile_pool(name="initp", bufs=2) as ipool:
                    c0 = 0
                    while c0 < n_s:
                        w = min(512, n_s - c0)
                        fch = ipool.tile([fdim, 512], F32, name="fch", tag="fch")
                        nc.sync.dma_start(out=fch[:, :w],
                                          in_=featT_dram[:, c0:c0 + w])
                        psi = ps_misc.tile([H, 512], F32, name="psi", tag="misc")
                        nc.tensor.matmul(out=psi[:, :w], lhsT=w_sb[:],
                                         rhs=fch[:, :w], start=True, stop=True)
                        nc.scalar.activation(out=hT[:, c0:c0 + w], in_=psi[:, :w],
                                             func=Act.Relu, bias=b_sb[:])
                        c0 += w

            emit_init(vfT, VF, V_S, wvar_sb, bvar_sb, hvT)
            emit_init(cfT, CF, C_S, wcon_sb, bcon_sb, hcT)

            qctr = [0]

            def emit_writeback(nblk, agin, tab, rstage):
                nc.sync.dma_start(
                    out=agin[0:nblk * P, 0:H].rearrange("(b p) f -> p b f", p=P),
                    in_=rstage[:, :nblk * H].rearrange("p (b f) -> p b f", f=H))
                nc.gpsimd.collective_compute(
                    "AllGather", mybir.AluOpType.bypass,
                    replica_groups=[list(range(CORES))],
                    ins=[agin[:]], outs=[tab[:]])

            def emit_shard_publish(hT, nblk, agin, tab):
                """initial publish: transpose all blocks then writeback."""
                rstage = row_pool.tile([P, NBU_V * H], BF16, name="rstage",
                                       tag="rstage")
                for b in range(nblk):
                    psr = ps_misc.tile([P, H], BF16, name="psr", tag="misc")
                    nc.tensor.transpose(out=psr[:], in_=hT[:, b * P:(b + 1) * P],
                                        identity=ident_sb[:])
                    nc.vector.tensor_copy(out=rstage[:, b * H:(b + 1) * H],
                                          in_=psr[:])
                emit_writeback(nblk, agin, tab, rstage)

            emit_shard_publish(hvT, NBU_V, agin_v, tabs_v[0])

            # ---- one message-passing phase
            def emit_phase(tab_src, wrows, idx_sb, pdst_sb, Tbw, nblk,
                           hT, W_sb, b_sb, writeback):
                nwin = Tbw.shape[1]
                grp_base = np.zeros((nblk, nwin), np.int64)
                grp_base.reshape(-1)[1:] = np.cumsum(Tbw.reshape(-1))[:-1]
                rstage = row_pool.tile([P, NBU_V * H], BF16, name="rstage",
                                       tag="rstage")
                for b in range(nblk):
                    nmm = int(Tbw[b].sum())
                    if nmm == 0:
                        continue
                    ps = ps_acc.tile([H, P], F32, name="ps", tag="ps")
                    mm = 0
                    for w in range(nwin):
                        Tg = int(Tbw[b, w])
                        base = int(grp_base[b, w])
                        done = 0
                        while done < Tg:
                            tc_ = min(TPC, Tg - done)
                            g = gpool.tile([P, TPC, H], BF16, name="g", tag="g")
                            _dma_gather_raw(
                                nc.gpsimd, g[:, :tc_, :],
                                tab_src[w * wrows:(w + 1) * wrows, 0:H],
                                idx_sb[:, (base + done) * 8:
                                       (base + done + tc_) * 8],
                                num_idxs=tc_ * P, elem_size=H, elem_step=ROWB,
                                queue_num=qctr[0] % 4)
                            qctr[0] += 1
                            for t in range(tc_):
                                gt = base + done + t
                                s2 = s2_pool.tile([P, P], BF16, name="s2",
                                                  tag="s2")
                                nc.vector.tensor_tensor(
                                    out=s2[:], in0=iota_sb[:],
                                    in1=pdst_sb[:, gt:gt + 1]
                                    .to_broadcast([P, P]),
                                    op=AluOp.is_equal)
                                nc.tensor.matmul(
                                    out=ps[:], lhsT=g[:, t, :], rhs=s2[:],
                                    start=(mm == 0), stop=(mm == nmm - 1))
                                mm += 1
                            done += tc_
                    # block update: hT = relu(hT + W.T @ msgT + b)
                    msg = msg_pool.tile([H, P], F32, name="msg", tag="msg")
                    nc.vector.tensor_copy(out=msg[:], in_=ps[:])
                    psu = ps_upd.tile([H, P], F32, name="psu", tag="psu")
                    nc.tensor.matmul(out=psu[:], lhsT=W_sb[:], rhs=msg[:],
                                     start=True, stop=True)
                    tmp = msg_pool.tile([H, P], F32, name="tmp", tag="tmp")
                    nc.vector.tensor_tensor(out=tmp[:], in0=psu[:],
                                            in1=hT[:, b * P:(b + 1) * P],
                                            op=AluOp.add)
                    nc.scalar.activation(out=hT[:, b * P:(b + 1) * P], in_=tmp[:],
                                         func=Act.Relu, bias=b_sb[:])
                    if writeback is not None:
                        psr = ps_misc.tile([P, H], BF16, name="psr", tag="misc")
                        nc.tensor.transpose(out=psr[:],
                                            in_=hT[:, b * P:(b + 1) * P],
                                            identity=ident_sb[:])
                        nc.vector.tensor_copy(out=rstage[:, b * H:(b + 1) * H],
                                              in_=psr[:])
                if writeback is not None:
                    writeback(rstage)

            for r in range(ROUNDS):
                emit_phase(tabs_v[r], VWROWS, idx_v2c_sb, pdst_v2c_sb, Tbw_c,
                           NBU_C, hcT, wv2c_sb, bv2c_sb,
                           lambda rs, r=r: emit_writeback(NBU_C, agin_c,
                                                          tabs_c[r], rs))
                last = r == ROUNDS - 1
                emit_phase(tabs_c[r], CWROWS, idx_c2v_sb, pdst_c2v_sb, Tbw_v,
                           NBU_V, hvT, wc2v_sb, bc2v_sb,
                           None if last else
                           (lambda rs, r=r: emit_writeback(NBU_V, agin_v,
                                                           tabs_v[r + 1], rs)))

            # ---- scores = h_var @ w_score + b_score (shard)
            c0 = 0
            while c0 < V_S:
                w = min(512, V_S - c0)
                pss = ps_misc.tile([1, 512], F32, name="pss", tag="misc")
                nc.tensor.matmul(out=pss[:, :w], lhsT=wsco_sb[:],
                                 rhs=hvT[:, c0:c0 + w], start=True, stop=True)
                sch = s2_pool.tile([1, 512], F32, name="sch", tag="sch")
                nc.vector.tensor_scalar(
                    out=sch[:, :w], in0=pss[:, :w],
                    scalar1=float(b_score_val), scalar2=None, op0=AluOp.add)
                nc.sync.dma_start(out=scores_out[None, c0:c0 + w],
                                  in_=sch[0:1, :w])
                c0 += 512

    nc.compile()
    return nc


_CACHE = {}


def kernel(**inputs):
    var_feat = np.asarray(inputs["var_feat"], np.float32)
    constr_feat = np.asarray(inputs["constr_feat"], np.float32)
    var_idx = np.asarray(inputs["var_idx"]).astype(np.int64)
    constr_idx = np.asarray(inputs["constr_idx"]).astype(np.int64)
    b_score_val = float(np.asarray(inputs["b_score"]).reshape(-1)[0])

    key = (var_idx.tobytes(), constr_idx.tobytes())
    if key in _CACHE:
        nc, idx_v, pdst_v, idx_c, pdst_c = _CACHE[key]
    else:
        # v2c: dst=constr, src=var
        idx_v, pdst_v, Tbw_c = _prep_direction(
            constr_idx, var_idx, C_CORE, NBU_C, VWIN, V_CORE, V_S)
        # c2v: dst=var, src=constr
        idx_c, pdst_c, Tbw_v = _prep_direction(
            var_idx, constr_idx, V_CORE, NBU_V, CWIN, C_CORE, C_S)
        nc = _build(Tbw_c, Tbw_v, b_score_val)
        _CACHE[key] = (nc, idx_v, pdst_v, idx_c, pdst_c)

    iota = np.broadcast_to(np.arange(P, dtype=np.float32),
                           (P, P)).astype(BF).copy()
    ident = np.eye(H, dtype=np.float32).astype(BF)

    vf_pad = np.zeros((CORES, V_S, VF), np.float32)
    vf_pad[:, :V_CORE] = var_feat.reshape(CORES, V_CORE, VF)
    cf_pad = np.zeros((CORES, C_S, CF), np.float32)
    cf_pad[:, :C_CORE] = constr_feat.reshape(CORES, C_CORE, CF)

    common = dict(
        wvar=np.ascontiguousarray(inputs["W_var"], dtype=np.float32),
        wcon=np.ascontiguousarray(inputs["W_con"], dtype=np.float32),
        wv2c=np.ascontiguousarray(inputs["W_v2c"], dtype=np.float32),
        wc2v=np.ascontiguousarray(inputs["W_c2v"], dtype=np.float32),
        wsco=np.ascontiguousarray(inputs["W_score"], dtype=np.float32).astype(BF),
        bvar=np.ascontiguousarray(inputs["b_var"], dtype=np.float32).reshape(H, 1),
        bcon=np.ascontiguousarray(inputs["b_con"], dtype=np.float32).reshape(H, 1),
        bv2c=np.ascontiguousarray(inputs["b_v2c"], dtype=np.float32).reshape(H, 1),
        bc2v=np.ascontiguousarray(inputs["b_c2v"], dtype=np.float32).reshape(H, 1),
        iota=iota, ident=ident,
    )
    in_maps = []
    for k in range(CORES):
        m = dict(common)
        m["vfT"] = np.ascontiguousarray(vf_pad[k].T)
        m["cfT"] = np.ascontiguousarray(cf_pad[k].T)
        m["idx_v2c"] = idx_v[k]
        m["pdst_v2c"] = pdst_v[k]
        m["idx_c2v"] = idx_c[k]
        m["pdst_c2v"] = pdst_c[k]
        in_maps.append(m)

    res = run_bass_kernel_spmd(nc, in_maps, list(range(CORES)))
    scores = np.concatenate([res.results[k]["scores"].reshape(-1)[:V_CORE]
                             for k in range(CORES)])
    return scores.astype(np.float32)


# revision 7
# speedup vs baseline: 3.4456x; 1.0731x over previous
"""Trainium2 Bass kernel: bipartite GNN message passing (BranchingGNN), 8-core SPMD.

Sharding: core k owns constraint rows [k*6250,(k+1)*6250) and variable rows
[k*12500,(k+1)*12500); each core processes all edges targeting its shard, so
messages need no cross-core reduction. Node tables live row-major in DRAM
(bf16 features in the first 128B of a 256B-strided row) and are re-broadcast
each phase by an AllGather of the updated shards.

Per phase (one message direction):
  - edges sorted by (dst-block, src-window, dst); per (block, window) group the
    raw edge list is cut into 128-edge tiles (up to 7 tiles per gather call).
    No slot padding and no dummy-row fetches: per-core shortfalls are trailing
    -1 indices, which the SWDGE ucode trims before descriptor generation.
  - dma_gather (custom emit: 128B rows at 256B stride) fetches source rows
    row-major [128 edges, 64] bf16.
  - per tile: DVE is_equal(iota, pdst) builds a one-hot S [128,128] bf16
    (pdst = within-block dst of each edge, -1 for pads -> zero row); one PE
    matmul (lhsT=g, rhs=S) accumulates the tile's segment-sum contribution
    into the block's PSUM [64,128] msgT.
  - per block: relu(h_prevT + W.T @ msgT + b) in transposed layout, PE
    transpose back to row-major into the writeback stage; after all blocks one
    DMA + AllGather republishes the updated shard.
"""
import sys

sys.path.insert(0, "/opt/trn_rl_repo")

import numpy as np
import ml_dtypes

import concourse.bass as bass
import concourse.bacc as bacc
import concourse.mybir as mybir
import concourse.tile as tile
from concourse.bass_utils import run_bass_kernel_spmd

# ---- problem constants
V, C, E = 100000, 50000, 1250000
VF, CF, H = 32, 32, 64
ROUNDS = 3
CORES = 8
P = 128
TPC = 7               # tiles per gather call (SWDGE ring cap)
ROWB = 128            # table row width in bf16 elems (64 data + 64 pad = 256B)

V_CORE, C_CORE = 12500, 6250          # real nodes per core
V_S, C_S = 12672, 6400                # shard rows (99 / 50 blocks)
NBU_V, NBU_C = 98, 49                 # updated blocks (last block stays zero)
RV, RC = CORES * V_S, CORES * C_S     # 101376 / 51200 table rows
VWIN, CWIN = 4, 2                     # source windows (int16 gather reach)
VWROWS, CWROWS = 2 * V_S, 4 * C_S     # 25344 / 25600 rows per window

BF16 = mybir.dt.bfloat16
F32 = mybir.dt.float32
I16 = mybir.dt.int16
BF = ml_dtypes.bfloat16


def _prep_direction(dst, src, n_dst_core, nblk, nwin, src_core, src_shard):
    """Per-direction metadata. Edges sorted by (core, block, window, dst);
    per (block, window) group cut into 128-edge tiles, per-core shortfall
    filled with trailing -1.

    Returns (idx_wrapped [CORES,128,T*8] int16, pdst [CORES,128,T] bf16,
    Tbw [nblk, nwin] int tile counts)."""
    dst = np.asarray(dst, np.int64)
    src = np.asarray(src, np.int64)
    per_win_ids = src_core * (CORES // nwin)

    core_of = dst // n_dst_core
    d_loc = dst % n_dst_core
    b_of = d_loc // P
    w_of = src // per_win_ids
    widx = (src % per_win_ids) // src_core * src_shard + src % src_core

    # sort by (core, block, window, dst)
    key = ((core_of * nblk + b_of) * nwin + w_of) * P + d_loc % P
    order = np.argsort(key, kind="stable")
    ks = key[order]
    widx_s = widx[order]
    pd_s = (d_loc % P)[order]

    counts = np.bincount(ks // P, minlength=CORES * nblk * nwin) \
        .reshape(CORES, nblk, nwin)
    Tbw = -(-counts.max(0) // P)                     # [nblk, nwin] max tiles
    Tbw = np.maximum(Tbw, (counts.max(0) > 0))       # 0 only if empty all cores

    grp_base = np.zeros((nblk, nwin), np.int64)      # tile base of group
    flat = Tbw.reshape(-1)
    grp_base.reshape(-1)[1:] = np.cumsum(flat)[:-1]
    Ttot = int(flat.sum())

    # position of each edge within its (core, b, w) group
    gk = ks // P                                     # core,b,w group id
    gcounts = np.bincount(gk, minlength=CORES * nblk * nwin)
    gstart = np.zeros(gcounts.size + 1, np.int64)
    gstart[1:] = np.cumsum(gcounts)
    rank = np.arange(dst.size, dtype=np.int64) - gstart[gk]

    c_s = gk // (nblk * nwin)
    bw = gk % (nblk * nwin)
    epos = grp_base.reshape(-1)[bw] * P + rank

    idx16 = np.full((CORES, Ttot * P), -1, np.int16)
    idx16[c_s, epos] = widx_s.astype(np.int16)

    # negative idxs crash the gather ucode, so every pad slot must carry a
    # valid row. Cycle each call's pads over that call's own valid idxs
    # (rows already being fetched, spread — no hot row); pdst stays -1 so
    # pads contribute nothing to the segment sum.
    flatT = Tbw.reshape(-1)
    flat_base = grp_base.reshape(-1)
    gcnt = counts  # [CORES, nblk, nwin]
    for bwi in np.nonzero(flatT > 0)[0]:
        base, T = int(flat_base[bwi]), int(flatT[bwi])
        b_, w_ = bwi // nwin, bwi % nwin
        for j0 in range(0, T, TPC):
            clen = min(TPC, T - j0) * P
            cpos = (base + j0) * P
            for k in range(CORES):
                valid = min(max(int(gcnt[k, b_, w_]) - j0 * P, 0), clen)
                if valid == clen:
                    continue
                if valid > 0:
                    src_slice = idx16[k, cpos:cpos + valid]
                    npad = clen - valid
                    reps = -(-npad // valid)
                    idx16[k, cpos + valid:cpos + clen] = \
                        np.tile(src_slice, reps)[:npad]
                else:
                    gbase = int(flat_base[bwi]) * P
                    gval = min(int(gcnt[k, b_, w_]), clen)
                    if gval == 0:
                        idx16[k, cpos:cpos + clen] = 0
                    else:
                        reps = -(-clen // gval)
                        idx16[k, cpos:cpos + clen] = \
                            np.tile(idx16[k, gbase:gbase + gval], reps)[:clen]
    pdst = np.full((CORES, Ttot * P), -1.0, np.float32)
    pdst[c_s, epos] = pd_s.astype(np.float32)
    pdst = pdst.reshape(CORES, Ttot, P).transpose(0, 2, 1)  # [CORES,128,T]

    # per-tile dst window [d0, d0+sw): S and the psum slice only need to
    # cover the dsts the tile actually touches (shared across cores).
    pd3 = pdst.transpose(0, 2, 1)                    # [CORES, Ttot, P]
    valid = pd3 >= 0
    dmin = np.where(valid, pd3, 999).min(axis=(0, 2))
    dmax = np.where(valid, pd3, -1).max(axis=(0, 2))
    d0 = np.minimum(dmin, P - 1).astype(np.int64)
    sw = np.maximum(dmax - d0 + 1, 1).astype(np.int64)
    # first tile of each block: full width (initializes the psum block)
    first = np.zeros(Ttot, bool)
    fb = 0
    for b in range(nblk):
        if Tbw[b].sum() > 0:
            first[fb] = True
        fb += int(Tbw[b].sum())
    d0[first] = 0
    sw[first] = P
    pdst_rel = pd3 - d0[None, :, None]
    pdst_rel[~valid] = -1
    pdst_rel = pdst_rel.transpose(0, 2, 1).astype(np.float32)  # [CORES,P,T]

    packed = np.zeros((CORES, P, Ttot * 8), np.int16)
    for k in range(CORES):
        a = idx16[k].reshape(-1, 16).T               # [16, Ttot*8]
        packed[k] = np.tile(a, (8, 1))
    return packed, pdst_rel.astype(BF), Tbw.astype(int), d0, sw


def _dma_gather_raw(gp, out_ap, in_ap, idxs_ap, num_idxs, elem_size, elem_step,
                    queue_num=0):
    """dma_gather (non-transpose, HBM source) allowing 128B rows at 256B stride."""
    from concourse import ap_utils
    gp._assert_queue_num(queue_num)
    assert idxs_ap.dtype == mybir.dt.int16
    assert in_ap.dtype == out_ap.dtype
    assert ap_utils.ap_is_contiguous(in_ap.ap[1:])
    assert ap_utils.ap_is_contiguous(out_ap.ap[1:])
    assert ap_utils.ap_is_contiguous(idxs_ap.ap[1:])
    assert in_ap.ap[-1][1] == out_ap.ap[-1][1] == elem_size
    assert out_ap.ap[0][1] * out_ap.ap[1][1] == num_idxs and num_idxs % 128 == 0
    assert in_ap.ap[0][0] == elem_step
    stride_bytes = elem_step * mybir.dt.size(in_ap.dtype)
    stride_bytes_256 = stride_bytes // 256
    assert stride_bytes_256 * 256 == stride_bytes and stride_bytes_256 < 256
    _in_ap = gp.lower_ap_dma(in_ap, for_custom_bir_dma=True)
    _idxs_ap = gp.lower_ap(idxs_ap)
    _out_ap = gp.lower_ap(out_ap)
    return gp.add_instruction(
        mybir.InstDMAGatherAnt(
            name=gp.bass.get_next_instruction_name(),
            ins=[*_in_ap, _idxs_ap, gp.lower_val_access(gp.to_reg(num_idxs))],
            outs=[_out_ap],
            transpose=False, num_idxs=num_idxs, elem_size=elem_size,
            stride_bytes_256=stride_bytes_256, gen_mode=0, single_packet=True,
            queue_num=queue_num, sbuf_tokens_per_rank=0,
            sbuf_free_dim_per_rank=0, sbuf_free_dim_pad_per_rank=0,
            sbuf_byte_offset=0))


def _build(Tbw_c, Tbw_v, d0_c, sw_c, d0_v, sw_v, b_score_val):
    """Build the shared SPMD program."""

    Tt_c, Tt_v = int(Tbw_c.sum()), int(Tbw_v.sum())

    nc = bacc.Bacc("TRN2", target_bir_lowering=False, num_devices=CORES,
                   num_swdge_queues=4)
    AluOp = mybir.AluOpType
    Act = mybir.ActivationFunctionType

    def ein(name, shape, dtype):
        return nc.dram_tensor(name, shape, dtype, kind="ExternalInput")

    vfT = ein("vfT", [VF, V_S], F32)
    cfT = ein("cfT", [CF, C_S], F32)
    wvar = ein("wvar", [VF, H], F32)
    wcon = ein("wcon", [CF, H], F32)
    wv2c = ein("wv2c", [H, H], F32)
    wc2v = ein("wc2v", [H, H], F32)
    wsco = ein("wsco", [H, 1], BF16)
    bvar = ein("bvar", [H, 1], F32)
    bcon = ein("bcon", [H, 1], F32)
    bv2c = ein("bv2c", [H, 1], F32)
    bc2v = ein("bc2v", [H, 1], F32)
    idx_v2c_d = ein("idx_v2c", [P, Tt_c * 8], I16)
    idx_c2v_d = ein("idx_c2v", [P, Tt_v * 8], I16)
    pdst_v2c_d = ein("pdst_v2c", [P, Tt_c], BF16)
    pdst_c2v_d = ein("pdst_c2v", [P, Tt_v], BF16)
    iota_d = ein("iota", [P, P], BF16)
    ident_d = ein("ident", [H, H], BF16)
    scores_out = nc.dram_tensor("scores", [V_S], F32, kind="ExternalOutput")

    with tile.TileContext(nc) as tc:
        with (
            tc.tile_pool(name="const", bufs=1) as cpool,
            tc.tile_pool(name="state", bufs=1) as spool,
            tc.tile_pool(name="dram", bufs=1, space="DRAM") as dpool,
            tc.tile_pool(name="gpool", bufs=24) as gpool,
            tc.tile_pool(name="s2p", bufs=6) as s2_pool,
            tc.tile_pool(name="msgp", bufs=3) as msg_pool,
            tc.tile_pool(name="rowp", bufs=2) as row_pool,
            tc.tile_pool(name="ps_acc", bufs=4, space="PSUM") as ps_acc,
            tc.tile_pool(name="ps_upd", bufs=2, space="PSUM") as ps_upd,
            tc.tile_pool(name="ps_misc", bufs=2, space="PSUM") as ps_misc,
        ):
            def load_const(name, dram, shape, dtype):
                t = cpool.tile(shape, dtype, name=name)
                nc.sync.dma_start(out=t[:], in_=dram[:])
                return t

            iota_sb = load_const("iota_sb", iota_d, [P, P], BF16)
            ident_sb = load_const("ident_sb", ident_d, [H, H], BF16)
            wvar_sb = load_const("wvar_sb", wvar, [VF, H], F32)
            wcon_sb = load_const("wcon_sb", wcon, [CF, H], F32)
            wv2c_sb = load_const("wv2c_sb", wv2c, [H, H], F32)
            wc2v_sb = load_const("wc2v_sb", wc2v, [H, H], F32)
            wsco_sb = load_const("wsco_sb", wsco, [H, 1], BF16)
            bvar_sb = load_const("bvar_sb", bvar, [H, 1], F32)
            bcon_sb = load_const("bcon_sb", bcon, [H, 1], F32)
            bv2c_sb = load_const("bv2c_sb", bv2c, [H, 1], F32)
            bc2v_sb = load_const("bc2v_sb", bc2v, [H, 1], F32)
            idx_v2c_sb = load_const("idx_v2c_sb", idx_v2c_d, [P, Tt_c * 8], I16)
            idx_c2v_sb = load_const("idx_c2v_sb", idx_c2v_d, [P, Tt_v * 8], I16)
            pdst_v2c_sb = load_const("pdst_v2c_sb", pdst_v2c_d, [P, Tt_c], BF16)
            pdst_c2v_sb = load_const("pdst_c2v_sb", pdst_c2v_d, [P, Tt_v], BF16)

            zrow_sb = cpool.tile([P, ROWB], BF16, name="zrow_sb")
            nc.vector.memset(zrow_sb[:], 0.0)

            # pre-zero gather buffers once: stale contents stay finite, so
            # S's zero rows always multiply finite values (no NaN*0).
            for _ in range(24):
                gz = gpool.tile([P, TPC, H], BF16, name="g", tag="g")
                nc.vector.memset(gz[:], 0.0)

            hvT = spool.tile([H, V_S], BF16, name="hvT")
            hcT = spool.tile([H, C_S], BF16, name="hcT")
            tabs_v = [dpool.tile([RV, ROWB], BF16, name=f"tab_v{i}",
                                 addr_space="Shared", tag=f"tab_v{i}")
                      for i in range(ROUNDS)]
            tabs_c = [dpool.tile([RC, ROWB], BF16, name=f"tab_c{i}",
                                 addr_space="Shared", tag=f"tab_c{i}")
                      for i in range(ROUNDS)]
            agin_v = dpool.tile([V_S, ROWB], BF16, name="agin_v")
            agin_c = dpool.tile([C_S, ROWB], BF16, name="agin_c")

            # zero the shard tail (pad rows shipped by every AllGather)
            nc.sync.dma_start(out=agin_v[NBU_V * P:V_S, :], in_=zrow_sb[:])
            nc.sync.dma_start(out=agin_c[NBU_C * P:C_S, :], in_=zrow_sb[:])

            # ---- initial embeddings hT = relu(W.T @ featT + b)
            def emit_init(featT_dram, fdim, n_s, w_sb, b_sb, hT):
                with tc.tile_pool(name="initp", bufs=2) as ipool:
                    c0 = 0
                    while c0 < n_s:
                        w = min(512, n_s - c0)
                        fch = ipool.tile([fdim, 512], F32, name="fch", tag="fch")
                        nc.sync.dma_start(out=fch[:, :w],
                                          in_=featT_dram[:, c0:c0 + w])
                        psi = ps_misc.tile([H, 512], F32, name="psi", tag="misc")
                        nc.tensor.matmul(out=psi[:, :w], lhsT=w_sb[:],
                                         rhs=fch[:, :w], start=True, stop=True)
                        nc.scalar.activation(out=hT[:, c0:c0 + w], in_=psi[:, :w],
                                             func=Act.Relu, bias=b_sb[:])
                        c0 += w

            emit_init(vfT, VF, V_S, wvar_sb, bvar_sb, hvT)
            emit_init(cfT, CF, C_S, wcon_sb, bcon_sb, hcT)

            qctr = [0]

            def emit_writeback(nblk, agin, tab, rstage):
                nc.sync.dma_start(
                    out=agin[0:nblk * P, 0:H].rearrange("(b p) f -> p b f", p=P),
                    in_=rstage[:, :nblk * H].rearrange("p (b f) -> p b f", f=H))
                nc.gpsimd.collective_compute(
                    "AllGather", mybir.AluOpType.bypass,
                    replica_groups=[list(range(CORES))],
                    ins=[agin[:]], outs=[tab[:]])

            def emit_shard_publish(hT, nblk, agin, tab):
                """initial publish: transpose all blocks then writeback."""
                rstage = row_pool.tile([P, NBU_V * H], BF16, name="rstage",
                                       tag="rstage")
                for b in range(nblk):
                    psr = ps_misc.tile([P, H], BF16, name="psr", tag="misc")
                    nc.tensor.transpose(out=psr[:], in_=hT[:, b * P:(b + 1) * P],
                                        identity=ident_sb[:])
                    nc.vector.tensor_copy(out=rstage[:, b * H:(b + 1) * H],
                                          in_=psr[:])
                emit_writeback(nblk, agin, tab, rstage)

            emit_shard_publish(hvT, NBU_V, agin_v, tabs_v[0])

            # ---- one message-passing phase
            def emit_phase(tab_src, wrows, idx_sb, pdst_sb, Tbw, d0a, swa,
                           nblk, hT, W_sb, b_sb, writeback):
                nwin = Tbw.shape[1]
                grp_base = np.zeros((nblk, nwin), np.int64)
                grp_base.reshape(-1)[1:] = np.cumsum(Tbw.reshape(-1))[:-1]
                rstage = row_pool.tile([P, NBU_V * H], BF16, name="rstage",
                                       tag="rstage")
                for b in range(nblk):
                    nmm = int(Tbw[b].sum())
                    if nmm == 0:
                        continue
                    ps = ps_acc.tile([H, P], F32, name="ps", tag="ps")
                    mm = 0
                    for w in range(nwin):
                        Tg = int(Tbw[b, w])
                        base = int(grp_base[b, w])
                        done = 0
                        while done < Tg:
                            tc_ = min(TPC, Tg - done)
                            g = gpool.tile([P, TPC, H], BF16, name="g", tag="g")
                            _dma_gather_raw(
                                nc.gpsimd, g[:, :tc_, :],
                                tab_src[w * wrows:(w + 1) * wrows, 0:H],
                                idx_sb[:, (base + done) * 8:
                                       (base + done + tc_) * 8],
                                num_idxs=tc_ * P, elem_size=H, elem_step=ROWB,
                                queue_num=qctr[0] % 4)
                            qctr[0] += 1
                            for t in range(tc_):
                                gt = base + done + t
                                d0_, sw_ = int(d0a[gt]), int(swa[gt])
                                s2 = s2_pool.tile([P, P], BF16, name="s2",
                                                  tag="s2")
                                nc.vector.tensor_tensor(
                                    out=s2[:, :sw_], in0=iota_sb[:, :sw_],
                                    in1=pdst_sb[:, gt:gt + 1]
                                    .to_broadcast([P, sw_]),
                                    op=AluOp.is_equal)
                                nc.tensor.matmul(
                                    out=ps[:, d0_:d0_ + sw_],
                                    lhsT=g[:, t, :], rhs=s2[:, :sw_],
                                    start=(mm == 0), stop=(mm == nmm - 1))
                                mm += 1
                            done += tc_
                    # block update: hT = relu(hT + W.T @ msgT + b)
                    msg = msg_pool.tile([H, P], F32, name="msg", tag="msg")
                    nc.vector.tensor_copy(out=msg[:], in_=ps[:])
                    psu = ps_upd.tile([H, P], F32, name="psu", tag="psu")
                    nc.tensor.matmul(out=psu[:], lhsT=W_sb[:], rhs=msg[:],
                                     start=True, stop=True)
                    tmp = msg_pool.tile([H, P], F32, name="tmp", tag="tmp")
                    nc.vector.tensor_tensor(out=tmp[:], in0=psu[:],
                                            in1=hT[:, b * P:(b + 1) * P],
                                            op=AluOp.add)
                    nc.scalar.activation(out=hT[:, b * P:(b + 1) * P], in_=tmp[:],
                                         func=Act.Relu, bias=b_sb[:])
                    if writeback is not None:
                        psr = ps_misc.tile([P, H], BF16, name="psr", tag="misc")
                        nc.tensor.transpose(out=psr[:],
                                            in_=hT[:, b * P:(b + 1) * P],
                                            identity=ident_sb[:])
                        nc.vector.tensor_copy(out=rstage[:, b * H:(b + 1) * H],
                                              in_=psr[:])
                if writeback is not None:
                    writeback(rstage)

            for r in range(ROUNDS):
                emit_phase(tabs_v[r], VWROWS, idx_v2c_sb, pdst_v2c_sb, Tbw_c,
                           d0_c, sw_c, NBU_C, hcT, wv2c_sb, bv2c_sb,
                           lambda rs, r=r: emit_writeback(NBU_C, agin_c,
                                                          tabs_c[r], rs))
                last = r == ROUNDS - 1
                emit_phase(tabs_c[r], CWROWS, idx_c2v_sb, pdst_c2v_sb, Tbw_v,
                           d0_v, sw_v, NBU_V, hvT, wc2v_sb, bc2v_sb,
                           None if last else
                           (lambda rs, r=r: emit_writeback(NBU_V, agin_v,
                                                           tabs_v[r + 1], rs)))

            # ---- scores = h_var @ w_score + b_score (shard)
            c0 = 0
            while c0 < V_S:
                w = min(512, V_S - c0)
                pss = ps_misc.tile([1, 512], F32, name="pss", tag="misc")
                nc.tensor.matmul(out=pss[:, :w], lhsT=wsco_sb[:],
                                 rhs=hvT[:, c0:c0 + w], start=True, stop=True)
                sch = s2_pool.tile([1, 512], F32, name="sch", tag="sch")
                nc.vector.tensor_scalar(
                    out=sch[:, :w], in0=pss[:, :w],
                    scalar1=float(b_score_val), scalar2=None, op0=AluOp.add)
                nc.sync.dma_start(out=scores_out[None, c0:c0 + w],
                                  in_=sch[0:1, :w])
                c0 += 512

    nc.compile()
    return nc


_CACHE = {}


def kernel(**inputs):
    var_feat = np.asarray(inputs["var_feat"], np.float32)
    constr_feat = np.asarray(inputs["constr_feat"], np.float32)
    var_idx = np.asarray(inputs["var_idx"]).astype(np.int64)
    constr_idx = np.asarray(inputs["constr_idx"]).astype(np.int64)
    b_score_val = float(np.asarray(inputs["b_score"]).reshape(-1)[0])

    key = (var_idx.tobytes(), constr_idx.tobytes())
    if key in _CACHE:
        nc, idx_v, pdst_v, idx_c, pdst_c = _CACHE[key]
    else:
        # v2c: dst=constr, src=var
        idx_v, pdst_v, Tbw_c, d0_c, sw_c = _prep_direction(
            constr_idx, var_idx, C_CORE, NBU_C, VWIN, V_CORE, V_S)
        # c2v: dst=var, src=constr
        idx_c, pdst_c, Tbw_v, d0_v, sw_v = _prep_direction(
            var_idx, constr_idx, V_CORE, NBU_V, CWIN, C_CORE, C_S)
        nc = _build(Tbw_c, Tbw_v, d0_c, sw_c, d0_v, sw_v, b_score_val)
        _CACHE[key] = (nc, idx_v, pdst_v, idx_c, pdst_c)

    iota = np.broadcast_to(np.arange(P, dtype=np.float32),
                           (P, P)).astype(BF).copy()
    ident = np.eye(H, dtype=np.float32).astype(BF)

    vf_pad = np.zeros((CORES, V_S, VF), np.float32)
    vf_pad[:, :V_CORE] = var_feat.reshape(CORES, V_CORE, VF)
    cf_pad = np.zeros((CORES, C_S, CF), np.float32)
    cf_pad[:, :C_CORE] = constr_feat.reshape(CORES, C_CORE, CF)

    common = dict(
        wvar=np.ascontiguousarray(inputs["W_var"], dtype=np.float32),
        wcon=np.ascontiguousarray(inputs["W_con"], dtype=np.float32),
        wv2c=np.ascontiguousarray(inputs["W_v2c"], dtype=np.float32),
        wc2v=np.ascontiguousarray(inputs["W_c2v"], dtype=np.float32),
        wsco=np.ascontiguousarray(inputs["W_score"], dtype=np.float32).astype(BF),
        bvar=np.ascontiguousarray(inputs["b_var"], dtype=np.float32).reshape(H, 1),
        bcon=np.ascontiguousarray(inputs["b_con"], dtype=np.float32).reshape(H, 1),
        bv2c=np.ascontiguousarray(inputs["b_v2c"], dtype=np.float32).reshape(H, 1),
        bc2v=np.ascontiguousarray(inputs["b_c2v"], dtype=np.float32).reshape(H, 1),
        iota=iota, ident=ident,
    )
    in_maps = []
    for k in range(CORES):
        m = dict(common)
        m["vfT"] = np.ascontiguousarray(vf_pad[k].T)
        m["cfT"] = np.ascontiguousarray(cf_pad[k].T)
        m["idx_v2c"] = idx_v[k]
        m["pdst_v2c"] = pdst_v[k]
        m["idx_c2v"] = idx_c[k]
        m["pdst_c2v"] = pdst_c[k]
        in_maps.append(m)

    res = run_bass_kernel_spmd(nc, in_maps, list(range(CORES)))
    scores = np.concatenate([res.results[k]["scores"].reshape(-1)[:V_CORE]
                             for k in range(CORES)])
    return scores.astype(np.float32)
